# revision 12
# baseline (speedup 1.0000x reference)
"""Trainium2 Bass kernel for nn_MultiHead (retrieval_knn).

Full inputs in, full output out. Internally: shard the N0 query axis across
8 NeuronCores, replicate the coarse stages; BN batch stats all-reduced.
"""

import numpy as np

import concourse.bass as bass
import concourse.bacc as bacc
import concourse.mybir as mybir
from concourse.tile import TileContext
from concourse.bass_utils import run_bass_kernel_spmd
from concourse.dve_spec import Spec, Src0, select, eq, scan, Idx, AluOp, lower, MaxNeg
import concourse.dve_ops as dve_ops
from concourse.dve_ops import DveOp, OPS, CUSTOM_DVE_SPECS
from concourse.dve_uop import DveOpSpec

F32 = mybir.dt.float32
I16 = mybir.dt.int16
AF = mybir.ActivationFunctionType

N_CORES = 8
N0_FULL = 98304
MS = [24576, 6144, 1536]          # candidate counts, stages 1..3
FDIMS = [64, 128, 256, 512]       # encoder dims, stages 0..3
DOUT = 64
DCLS = 128
KCLS = 17
EPS = 1e-5
CH = 512                          # matmul moving chunk


# ---------------------------------------------------------------- custom DVE op
def _register_argmax_op():
    """Fused single-pass argmax: feed the stream REVERSED; accum_out is the
    reversed-stream position of the last max attainment == first forward
    occurrence of the max.  idx_fwd = (N-1) - accum."""
    for op in OPS:
        if op.name == "ARGMAX_REV_ANT":
            return op

    FLT_MAX = np.float32(3.4028235e38)

    def _ref(in0, in1, s0, s1, imm2):
        r = np.maximum.accumulate(in0, axis=-1)
        out = np.where(
            in0 == r, np.arange(in0.shape[-1], dtype=np.float32), -FLT_MAX
        )
        acc = out.max(axis=-1, keepdims=True)
        return out, acc

    spec = Spec(
        body=select(eq(scan(AluOp.MAX, Src0), Src0), Idx, MaxNeg),
        accum=AluOp.MAX,
        reference=_ref,
    )
    op = DveOp("ARGMAX_REV_ANT", spec, subdim=False, uops_sha={})
    for ver in ("v3", "v4"):
        s = DveOpSpec(name=op.name, opcode=0, uops=lower(spec, ver=ver), rd1_en=False)
        op.uops_sha[ver] = s.sha(ver)
    OPS.append(op)
    CUSTOM_DVE_SPECS[op.name] = spec
    dve_ops._SUB_OPCODE_FOR_NAME[op.name] = dve_ops._CUSTOM_DVE_ROW_BASE + len(OPS) - 1
    assert max(dve_ops._SUB_OPCODE_FOR_NAME.values()) < 0x20
    return op


def _pack_factor(m, unit=CH):
    # pack [6, M] -> groups of 6 rows at partition 32*g, so the SBUF
    # byte-range allocation stays small; engine ops must start at 0/32/64/96
    for p in (4, 2, 1):
        if m % p == 0 and (m // p) % unit == 0:
            return p
    return 1


# ---------------------------------------------------------------- builder
def build_nc(qpc=N0_FULL // N_CORES, ms=tuple(MS), n0_full=N0_FULL):
    """Build the SPMD Bass program for one core (same NEFF on all cores)."""
    argmax_op = _register_argmax_op()
    ms = list(ms)
    n_qt = qpc // 128
    assert qpc % CH == 0 and qpc % 128 == 0
    for m in ms:
        assert m % CH == 0

    packs = [_pack_factor(m) for m in ms]
    qpack = _pack_factor(qpc, unit=128)
    qw = qpc // qpack
    assert qw % 128 == 0

    def rows(p):
        return 32 * (p - 1) + 6

    nc = bacc.Bacc("TRN2", target_bir_lowering=False, debug=False,
                   num_devices=N_CORES)

    # ---- I/O -------------------------------------------------------------
    def din(name, shape):
        return nc.dram_tensor(name, shape, F32, kind="ExternalInput").ap()

    qaug_in = din("qaug", [6, qpc])
    caug_in = [din(f"caug{s}", [rows(packs[s]), ms[s] // packs[s]])
               for s in range(3)]
    x0T = din("x0T", [64, qpc])
    xT = [din(f"x{s+1}T", [FDIMS[s + 1], ms[s]]) for s in range(3)]
    sw = [din(f"sw{i}", [FDIMS[i], DOUT]) for i in range(4)]
    sg = [din(f"sg{i}", [DOUT, 1]) for i in range(4)]
    sbeta = [din(f"sbeta{i}", [DOUT, 1]) for i in range(4)]
    cw1 = din("cw1", [4 * DOUT, DCLS])
    cg1 = din("cg1", [DCLS, 1])
    cbeta1 = din("cbeta1", [DCLS, 1])
    cw2 = din("cw2", [DCLS, KCLS])
    b2rep = din("b2rep", [128, KCLS])

    out_d = nc.dram_tensor("out", [qpc, KCLS], F32, kind="ExternalOutput").ap()

    # ---- internal DRAM ---------------------------------------------------
    f0d = nc.dram_tensor("f0d", [DOUT, qpc], F32).ap()
    fd = [nc.dram_tensor(f"f{s+1}d", [DOUT, ms[s]], F32).ap() for s in range(3)]
    gd = [nc.dram_tensor(f"g{s+1}d", [DOUT, qpc], F32).ap() for s in range(3)]
    idxd = [nc.dram_tensor(f"idx{s+1}d", [qpc], I16).ap() for s in range(3)]
    cc0in = nc.dram_tensor("cc0in", [1, 2 * DOUT], F32).ap()
    cc0out = nc.dram_tensor("cc0out", [1, 2 * DOUT], F32, addr_space="Shared").ap()
    cc1in = nc.dram_tensor("cc1in", [1, 2 * DCLS], F32).ap()
    cc1out = nc.dram_tensor("cc1out", [1, 2 * DCLS], F32, addr_space="Shared").ap()

    rg = [list(range(N_CORES))]
    ax = mybir.AxisListType.X
    AOT = mybir.AluOpType

    def bn_scale_shift(pool, pfx, d, gsums, gsumsq, n_all, g_in, beta_in):
        """-> (scale, shift) [d,1] tiles: scale = g*rsqrt(var+eps),
        shift = beta - mean*scale."""
        mean = pool.tile([d, 1], F32, tag=f"{pfx}mean")
        var = pool.tile([d, 1], F32, tag=f"{pfx}var")
        nc.vector.tensor_scalar_mul(mean[:], gsums[:], 1.0 / n_all)
        nc.vector.tensor_scalar_mul(var[:], gsumsq[:], 1.0 / n_all)
        m2 = pool.tile([d, 1], F32, tag=f"{pfx}m2")
        nc.vector.tensor_mul(m2[:], mean[:], mean[:])
        nc.vector.tensor_sub(var[:], var[:], m2[:])
        nc.vector.tensor_scalar_add(var[:], var[:], EPS)
        r0 = pool.tile([d, 1], F32, tag=f"{pfx}r0")
        nc.vector.reciprocal(r0[:], var[:])
        rst = pool.tile([d, 1], F32, tag=f"{pfx}rst")
        nc.scalar.activation(rst[:], r0[:], AF.Sqrt)
        t0 = pool.tile([d, 1], F32, tag=f"{pfx}t0")
        nc.vector.tensor_mul(t0[:], rst[:], rst[:])
        nc.vector.tensor_mul(t0[:], t0[:], var[:])
        nc.vector.tensor_scalar(t0[:], t0[:], -0.5, 1.5, op0=AOT.mult, op1=AOT.add)
        nc.vector.tensor_mul(rst[:], rst[:], t0[:])
        gsb = pool.tile([d, 1], F32, tag=f"{pfx}gsb")
        bsb = pool.tile([d, 1], F32, tag=f"{pfx}bsb")
        nc.sync.dma_start(gsb[:], g_in[:])
        nc.sync.dma_start(bsb[:], beta_in[:])
        scale = pool.tile([d, 1], F32, tag=f"{pfx}scale")
        nc.vector.tensor_mul(scale[:], rst[:], gsb[:])
        shift = pool.tile([d, 1], F32, tag=f"{pfx}shift")
        nc.vector.tensor_mul(shift[:], mean[:], scale[:])
        nc.vector.tensor_sub(shift[:], bsb[:], shift[:])
        return scale, shift

    with TileContext(nc) as tc:
        with tc.tile_pool(name="persist", bufs=1) as pp:
            idx16 = pp.tile([128, 3 * n_qt], I16, tag="idx16")

            # ================= phase A: stage features (1,2,3 then 0) ======
            def stage_feats(s, src, npts, n_all, w_in, g_in, beta_in, dst,
                            fpool, partial_cc=None):
                fdim = FDIMS[s]
                nk = max(1, fdim // 128)
                kp = fdim // nk
                nch = npts // CH
                with tc.tile_pool(name=f"stA{s}", bufs=3) as sp, \
                     tc.tile_pool(name=f"stApsum{s}", bufs=4, space="PSUM") as psp:
                    wt = []
                    for k in range(nk):
                        w = sp.tile([kp, DOUT], F32, tag="w")
                        nc.sync.dma_start(w[:], w_in[k * kp:(k + 1) * kp, :])
                        wt.append(w)
                    fsb = fpool.tile([DOUT, npts], F32, tag=f"fsb{s}")
                    scol = sp.tile([DOUT, nch], F32, tag="scol")
                    sqcol = sp.tile([DOUT, nch], F32, tag="sqcol")
                    for ci in range(nch):
                        ps = psp.tile([DOUT, CH], F32, tag="ps")
                        for k in range(nk):
                            xt = sp.tile([kp, CH], F32, tag="xt")
                            nc.sync.dma_start(
                                xt[:], src[k * kp:(k + 1) * kp,
                                           ci * CH:(ci + 1) * CH])
                            nc.tensor.matmul(ps[:], wt[k][:], xt[:],
                                             start=(k == 0), stop=(k == nk - 1))
                        nc.scalar.activation(
                            fsb[:, ci * CH:(ci + 1) * CH], ps[:], AF.Copy,
                            accum_out=scol[:, ci:ci + 1])
                        sq = sp.tile([DOUT, CH], F32, tag="sq")
                        nc.scalar.activation(
                            sq[:], ps[:], AF.Square,
                            accum_out=sqcol[:, ci:ci + 1])
                    sums = sp.tile([DOUT, 1], F32, tag="sums")
                    sumsq = sp.tile([DOUT, 1], F32, tag="sumsq")
                    nc.vector.reduce_sum(sums[:], scol[:], axis=ax)
                    nc.vector.reduce_sum(sumsq[:], sqcol[:], axis=ax)

                    def finish(gsums, gsumsq, fin_pool):
                        scale, shift = bn_scale_shift(
                            fin_pool, f"s{s}", DOUT, gsums, gsumsq, n_all,
                            g_in, beta_in)
                        nc.scalar.activation(fsb[:], fsb[:], AF.Relu,
                                             bias=shift[:], scale=scale[:])
                        nc.sync.dma_start(dst[:], fsb[:])

                    if partial_cc is None:
                        with tc.tile_pool(name=f"stAF{s}", bufs=1) as fp2:
                            finish(sums, sumsq, fp2)
                        return None
                    nc.sync.dma_start(partial_cc[0, 0:DOUT], sums[:])
                    nc.sync.dma_start(partial_cc[0, DOUT:2 * DOUT], sumsq[:])
                    return finish

            # stages 1..3 replicated, local stats; each scope frees its fsb
            for s in range(3):
                with tc.tile_pool(name=f"fsbp{s+1}", bufs=1) as fpool:
                    stage_feats(s + 1, xT[s], ms[s], ms[s], sw[s + 1],
                                sg[s + 1], sbeta[s + 1], fd[s], fpool)
            # stage 0 (sharded): partial stats + allreduce, then finish
            with tc.tile_pool(name="fsbp0", bufs=1) as fpool0:
                fin0 = stage_feats(0, x0T, qpc, n0_full, sw[0], sg[0],
                                   sbeta[0], f0d, fpool0, partial_cc=cc0in)
                nc.gpsimd.collective_compute(
                    "AllReduce", AOT.add, replica_groups=rg,
                    ins=[cc0in[:]], outs=[cc0out[:]])
                with tc.tile_pool(name="st0fin", bufs=1) as zp:
                    gs = zp.tile([DOUT, 1], F32, tag="gs")
                    gq = zp.tile([DOUT, 1], F32, tag="gq")
                    nc.sync.dma_start(
                        gs[:], bass.AP(cc0out.tensor, 0, [[1, DOUT], [1, 1]]))
                    nc.sync.dma_start(
                        gq[:], bass.AP(cc0out.tensor, DOUT, [[1, DOUT], [1, 1]]))
                    fin0(gs, gq, zp)

            # ================= phase B: kNN argmin ==========================
            m23 = ms[1] + ms[2]
            with tc.tile_pool(name="tbl", bufs=1) as tp:
                caug_sb = []
                for s in range(3):
                    t = tp.tile([rows(packs[s]), ms[s] // packs[s]], F32,
                                tag=f"caug{s}")
                    nc.sync.dma_start(t[:], caug_in[s][:])
                    caug_sb.append(t)
                # rows 32g+3..32g+5 arrive as copies of xyz; square via a
                # scratch tile (engine ops can't start at partition 32g+3)
                wmax = max(ms[s] // packs[s] for s in range(3))
                with tc.tile_pool(name="sqscr", bufs=1) as qsp:
                    scr = qsp.tile([128, wmax], F32, tag="scr")
                    for s in range(3):
                        w = ms[s] // packs[s]
                        for g in range(packs[s]):
                            nc.scalar.activation(
                                scr[32 * g:32 * g + 3, :w],
                                caug_sb[s][32 * g:32 * g + 3, :], AF.Square)
                            nc.sync.dma_start(
                                caug_sb[s][32 * g + 3:32 * g + 6, :],
                                scr[32 * g:32 * g + 3, :w])

                maxp = max(packs)
                with tc.tile_pool(name="knn", bufs=1) as kp_, \
                     tc.tile_pool(name="stg", bufs=2) as sgp, \
                     tc.tile_pool(name="knnpsum", bufs=2, space="PSUM") as kpp:
                    vals1 = kp_.tile([128, ms[0]], F32, tag="vals1")
                    vals23 = kp_.tile([128, m23], F32, tag="vals23")
                    idxbuf = kp_.tile([128, 3 * n_qt], F32, tag="idxbuf")
                    for qt in range(n_qt):
                        # stage the query tile at partition bases 0/32/64/96
                        stg = sgp.tile([32 * (maxp - 1) + 6, 128], F32, tag="stg")
                        for g in range(maxp):
                            nc.sync.dma_start(
                                stg[32 * g:32 * g + 6, :],
                                qaug_in[0:6, qt * 128:(qt + 1) * 128])
                        for s in range(3):
                            p = packs[s]
                            w = ms[s] // p
                            tpg = w // CH
                            base = 0 if s < 2 else ms[1]
                            dv = vals1 if s == 0 else vals23
                            if p > 1:
                                # p group-matmuls -> one psum tile -> one
                                # strided ACT copy into global-order columns
                                for t in range(tpg):
                                    ps = kpp.tile([128, p * CH], F32, tag="ps")
                                    for g in range(p):
                                        nc.tensor.matmul(
                                            ps[:, g * CH:(g + 1) * CH],
                                            stg[32 * g:32 * g + 6, :],
                                            caug_sb[s][32 * g:32 * g + 6,
                                                       t * CH:(t + 1) * CH],
                                            start=True, stop=True,
                                            tile_position=(32 * g, 0))
                                    dst = bass.AP(
                                        dv.tensor,
                                        dv.offset + base + t * CH,
                                        [dv.ap[0], [w, p], [1, CH]])
                                    nc.scalar.activation(dst, ps[:], AF.Copy)
                            else:
                                # batch consecutive tiles, contiguous copy
                                bt = 4
                                for t0 in range(0, tpg, bt):
                                    nb = min(bt, tpg - t0)
                                    ps = kpp.tile([128, nb * CH], F32, tag="ps")
                                    for j in range(nb):
                                        nc.tensor.matmul(
                                            ps[:, j * CH:(j + 1) * CH],
                                            stg[0:6, :],
                                            caug_sb[s][0:6,
                                                       (t0 + j) * CH:
                                                       (t0 + j + 1) * CH],
                                            start=True, stop=True,
                                            tile_position=(0, 0))
                                    co = base + t0 * CH
                                    nc.scalar.activation(
                                        dv[:, co:co + nb * CH], ps[:], AF.Copy)
                        views = [vals1[:, :],
                                 vals23[:, 0:ms[1]],
                                 vals23[:, ms[1]:m23]]
                        for s in range(3):
                            acc = kp_.tile([128, 1], F32, tag=f"acc{s}")
                            v = views[s]
                            nc.vector._custom_dve(argmax_op, out=v[:, ::-1],
                                                  in0=v[:, ::-1],
                                                  accum_out=acc[:])
                            nc.vector.tensor_scalar(
                                idxbuf[:, s * n_qt + qt:s * n_qt + qt + 1],
                                acc[:], -1.0, float(ms[s] - 1),
                                op0=AOT.mult, op1=AOT.add)
                    nc.vector.tensor_copy(idx16[:], idxbuf[:])

            # ================= phase C: gather + classifier =================
            for s in range(3):
                nc.sync.dma_start(
                    bass.AP(idxd[s].tensor, 0, [[1, 128], [128, n_qt]]),
                    idx16[:, s * n_qt:(s + 1) * n_qt])
            for s in range(3):
                with tc.tile_pool(name=f"gat{s}", bufs=1) as gp:
                    fsb = gp.tile([DOUT, ms[s]], F32, tag="gfsb")
                    nc.sync.dma_start(fsb[:], fd[s][:])
                    wr = gp.tile([DOUT, qpc // 16], I16, tag="wr")
                    for k in range(4):
                        nc.sync.dma_start(
                            wr[16 * k:16 * k + 16, :],
                            bass.AP(idxd[s].tensor, 0, [[1, 16], [16, qpc // 16]]))
                    gsb = gp.tile([DOUT, qpc], F32, tag="ggsb")
                    nc.gpsimd.ap_gather(gsb[:], fsb[:], wr[:],
                                        channels=DOUT, num_elems=ms[s],
                                        d=1, num_idxs=qpc)
                    nc.sync.dma_start(gd[s][:], gsb[:])

            nchq = qpc // CH
            with tc.tile_pool(name="cls", bufs=3) as cp, \
                 tc.tile_pool(name="clspsum", bufs=4, space="PSUM") as cpp:
                w1t = []
                for k in range(2):
                    w = cp.tile([128, DCLS], F32, tag="w1t")
                    nc.sync.dma_start(w[:], cw1[k * 128:(k + 1) * 128, :])
                    w1t.append(w)
                hsb = cp.tile([DCLS, qpc], F32, tag="hsb")
                scol = cp.tile([DCLS, nchq], F32, tag="cscol")
                sqcol = cp.tile([DCLS, nchq], F32, tag="csqcol")
                for ci in range(nchq):
                    ps = cpp.tile([DCLS, CH], F32, tag="cps")
                    for k in range(2):
                        xt = cp.tile([128, CH], F32, tag="cxt")
                        lo, hi = (f0d, gd[0]) if k == 0 else (gd[1], gd[2])
                        nc.sync.dma_start(xt[0:64, :],
                                          lo[:, ci * CH:(ci + 1) * CH])
                        nc.sync.dma_start(xt[64:128, :],
                                          hi[:, ci * CH:(ci + 1) * CH])
                        nc.tensor.matmul(ps[:], w1t[k][:], xt[:],
                                         start=(k == 0), stop=(k == 1))
                    nc.scalar.activation(hsb[:, ci * CH:(ci + 1) * CH], ps[:],
                                         AF.Copy, accum_out=scol[:, ci:ci + 1])
                    sq = cp.tile([DCLS, CH], F32, tag="csq")
                    nc.scalar.activation(sq[:], ps[:], AF.Square,
                                         accum_out=sqcol[:, ci:ci + 1])
                csums = cp.tile([DCLS, 1], F32, tag="csums")
                csumsq = cp.tile([DCLS, 1], F32, tag="csumsq")
                nc.vector.reduce_sum(csums[:], scol[:], axis=ax)
                nc.vector.reduce_sum(csumsq[:], sqcol[:], axis=ax)
                nc.sync.dma_start(cc1in[0, 0:DCLS], csums[:])
                nc.sync.dma_start(cc1in[0, DCLS:2 * DCLS], csumsq[:])
                nc.gpsimd.collective_compute(
                    "AllReduce", AOT.add, replica_groups=rg,
                    ins=[cc1in[:]], outs=[cc1out[:]])
                gs = cp.tile([DCLS, 1], F32, tag="cgs")
                gq = cp.tile([DCLS, 1], F32, tag="cgq")
                nc.sync.dma_start(gs[:], bass.AP(cc1out.tensor, 0,
                                                 [[1, DCLS], [1, 1]]))
                nc.sync.dma_start(gq[:], bass.AP(cc1out.tensor, DCLS,
                                                 [[1, DCLS], [1, 1]]))
                scale, shift = bn_scale_shift(cp, "c", DCLS, gs, gq, n0_full,
                                              cg1, cbeta1)
                nc.scalar.activation(hsb[:], hsb[:], AF.Relu,
                                     bias=shift[:], scale=scale[:])
                w2t = cp.tile([DCLS, KCLS], F32, tag="w2t")
                nc.sync.dma_start(w2t[:], cw2[:])
                b2t = cp.tile([128, KCLS], F32, tag="b2t")
                nc.sync.dma_start(b2t[:], b2rep[:])
                for qt in range(n_qt):
                    po = cpp.tile([128, KCLS], F32, tag="po")
                    nc.tensor.matmul(po[:], hsb[:, qt * 128:(qt + 1) * 128],
                                     w2t[:], start=True, stop=True)
                    ot = cp.tile([128, KCLS], F32, tag="ot")
                    nc.vector.tensor_add(ot[:], po[:], b2t[:])
                    nc.sync.dma_start(out_d[qt * 128:(qt + 1) * 128, :], ot[:])

    nc.compile()
    return nc


# ---------------------------------------------------------------- host side
def prep_inmaps(inputs, qpc=N0_FULL // N_CORES, ms=tuple(MS)):
    ms = list(ms)
    p0 = np.asarray(inputs["p0"], np.float32)
    x0 = np.asarray(inputs["x0"], np.float32)
    ps = [np.asarray(inputs[f"p{s+1}"], np.float32) for s in range(3)]
    xs = [np.asarray(inputs[f"x{s+1}"], np.float32) for s in range(3)]
    sw = [np.asarray(w, np.float32) for w in inputs["stage_w"]]
    sg = [np.asarray(w, np.float32).reshape(-1, 1) for w in inputs["stage_g"]]
    sbeta = [np.asarray(w, np.float32).reshape(-1, 1)
             for w in inputs["stage_beta"]]
    w1 = np.asarray(inputs["w1"], np.float32)
    g1 = np.asarray(inputs["g1"], np.float32).reshape(-1, 1)
    beta1 = np.asarray(inputs["beta1"], np.float32).reshape(-1, 1)
    w2 = np.asarray(inputs["w2"], np.float32)
    b2 = np.asarray(inputs["b2"], np.float32)

    packs = [_pack_factor(m) for m in ms]
    qpack = _pack_factor(qpc, unit=128)

    def pack6(a6, p):
        # [6, M] -> [32(p-1)+6, M/p]; group g's 6 rows at partition 32g
        m = a6.shape[1]
        w = m // p
        z = np.zeros((32 * (p - 1) + 6, w), np.float32)
        for g in range(p):
            z[32 * g:32 * g + 6, :] = a6[:, g * w:(g + 1) * w]
        return z

    shared = {}
    for s in range(3):
        a = np.concatenate([ps[s].T, ps[s].T], axis=0).astype(np.float32)
        shared[f"caug{s}"] = pack6(a, packs[s])
        shared[f"x{s+1}T"] = np.ascontiguousarray(xs[s].T)
    for i in range(4):
        shared[f"sw{i}"] = sw[i]
        shared[f"sg{i}"] = sg[i]
        shared[f"sbeta{i}"] = sbeta[i]
    shared["cw1"] = w1
    shared["cg1"] = g1
    shared["cbeta1"] = beta1
    shared["cw2"] = w2
    shared["b2rep"] = np.ascontiguousarray(
        np.broadcast_to(b2.reshape(1, -1), (128, b2.shape[0])))

    in_maps = []
    for c in range(N_CORES):
        sl = slice(c * qpc, (c + 1) * qpc)
        m = dict(shared)
        m["qaug"] = np.ascontiguousarray(np.concatenate(
            [p0[sl].T, np.full((3, qpc), -0.5, np.float32)], axis=0))
        m["x0T"] = np.ascontiguousarray(x0[sl].T)
        in_maps.append(m)
    return in_maps


_NC_CACHE = {}


def kernel(**inputs):
    key = "full"
    if key not in _NC_CACHE:
        _NC_CACHE[key] = build_nc()
    nc = _NC_CACHE[key]
    in_maps = prep_inmaps(inputs)
    res = run_bass_kernel_spmd(nc, in_maps, list(range(N_CORES)))
    out = np.concatenate([res.results[c]["out"] for c in range(N_CORES)], axis=0)
    return np.ascontiguousarray(out.astype(np.float32))


# revision 15
# speedup vs baseline: 1.4601x; 1.4601x over previous
"""Trainium2 Bass kernel for nn_MultiHead (retrieval_knn).

Full inputs in, full output out. Internally: shard the N0 query axis across
8 NeuronCores, replicate the coarse stages; BN batch stats all-reduced.
"""

import numpy as np

import concourse.bass as bass
import concourse.bacc as bacc
import concourse.mybir as mybir
from concourse.tile import TileContext
from concourse.bass_utils import run_bass_kernel_spmd
from concourse.dve_spec import Spec, Src0, select, eq, scan, Idx, AluOp, lower, MaxNeg
import concourse.dve_ops as dve_ops
from concourse.dve_ops import DveOp, OPS, CUSTOM_DVE_SPECS
from concourse.dve_uop import DveOpSpec

F32 = mybir.dt.float32
I16 = mybir.dt.int16
AF = mybir.ActivationFunctionType

N_CORES = 8
N0_FULL = 98304
MS = [24576, 6144, 1536]          # candidate counts, stages 1..3
FDIMS = [64, 128, 256, 512]       # encoder dims, stages 0..3
DOUT = 64
DCLS = 128
KCLS = 17
EPS = 1e-5
CH = 512                          # matmul moving chunk


# ---------------------------------------------------------------- custom DVE op
def _register_argmax_op():
    """Fused single-pass argmax: feed the stream REVERSED; accum_out is the
    reversed-stream position of the last max attainment == first forward
    occurrence of the max.  idx_fwd = (N-1) - accum."""
    for op in OPS:
        if op.name == "ARGMAX_REV_ANT":
            return op

    FLT_MAX = np.float32(3.4028235e38)

    def _ref(in0, in1, s0, s1, imm2):
        r = np.maximum.accumulate(in0, axis=-1)
        out = np.where(
            in0 == r, np.arange(in0.shape[-1], dtype=np.float32), -FLT_MAX
        )
        acc = out.max(axis=-1, keepdims=True)
        return out, acc

    spec = Spec(
        body=select(eq(scan(AluOp.MAX, Src0), Src0), Idx, MaxNeg),
        accum=AluOp.MAX,
        reference=_ref,
    )
    op = DveOp("ARGMAX_REV_ANT", spec, subdim=False, uops_sha={})
    for ver in ("v3", "v4"):
        s = DveOpSpec(name=op.name, opcode=0, uops=lower(spec, ver=ver), rd1_en=False)
        op.uops_sha[ver] = s.sha(ver)
    OPS.append(op)
    CUSTOM_DVE_SPECS[op.name] = spec
    dve_ops._SUB_OPCODE_FOR_NAME[op.name] = dve_ops._CUSTOM_DVE_ROW_BASE + len(OPS) - 1
    assert max(dve_ops._SUB_OPCODE_FOR_NAME.values()) < 0x20
    return op


def _pack_factor(m, unit=CH):
    # pack [6, M] -> groups of 6 rows at partition 32*g, so the SBUF
    # byte-range allocation stays small; engine ops must start at 0/32/64/96
    for p in (4, 2, 1):
        if m % p == 0 and (m // p) % unit == 0:
            return p
    return 1


# ---------------------------------------------------------------- builder
def build_nc(qpc=N0_FULL // N_CORES, ms=tuple(MS), n0_full=N0_FULL,
             collectives=True):
    """Build the SPMD Bass program for one core (same NEFF on all cores)."""
    argmax_op = _register_argmax_op()
    ms = list(ms)
    n_qt = qpc // 128
    assert qpc % CH == 0 and qpc % 128 == 0
    for m in ms:
        assert m % CH == 0

    packs = [_pack_factor(m) for m in ms]
    qpack = _pack_factor(qpc, unit=128)
    qw = qpc // qpack
    assert qw % 128 == 0

    def rows(p):
        return 32 * (p - 1) + 6

    nc = bacc.Bacc("TRN2", target_bir_lowering=False, debug=False,
                   num_devices=N_CORES)

    # ---- I/O -------------------------------------------------------------
    def din(name, shape):
        return nc.dram_tensor(name, shape, F32, kind="ExternalInput").ap()

    qaug_in = din("qaug", [6, qpc])
    caug_in = [din(f"caug{s}", [rows(packs[s]), ms[s] // packs[s]])
               for s in range(3)]
    x0T = din("x0T", [64, qpc])
    xT = [din(f"x{s+1}T", [FDIMS[s + 1], ms[s]]) for s in range(3)]
    sw = [din(f"sw{i}", [FDIMS[i], DOUT]) for i in range(4)]
    sg = [din(f"sg{i}", [DOUT, 1]) for i in range(4)]
    sbeta = [din(f"sbeta{i}", [DOUT, 1]) for i in range(4)]
    cw1 = din("cw1", [4 * DOUT, DCLS])
    cg1 = din("cg1", [DCLS, 1])
    cbeta1 = din("cbeta1", [DCLS, 1])
    cw2 = din("cw2", [DCLS, KCLS])
    b2rep = din("b2rep", [128, KCLS])

    out_d = nc.dram_tensor("out", [qpc, KCLS], F32, kind="ExternalOutput").ap()

    # ---- internal DRAM ---------------------------------------------------
    f0d = nc.dram_tensor("f0d", [DOUT, qpc], F32).ap()
    fd = [nc.dram_tensor(f"f{s+1}d", [DOUT, ms[s]], F32).ap() for s in range(3)]
    gd = [nc.dram_tensor(f"g{s+1}d", [DOUT, qpc], F32).ap() for s in range(3)]
    idxd = [nc.dram_tensor(f"idx{s+1}d", [qpc], I16).ap() for s in range(3)]
    cc0in = nc.dram_tensor("cc0in", [1, 2 * DOUT], F32).ap()
    cc0out = nc.dram_tensor("cc0out", [1, 2 * DOUT], F32, addr_space="Shared").ap()
    cc1in = nc.dram_tensor("cc1in", [1, 2 * DCLS], F32).ap()
    cc1out = nc.dram_tensor("cc1out", [1, 2 * DCLS], F32, addr_space="Shared").ap()

    rg = [list(range(N_CORES))]
    ax = mybir.AxisListType.X
    AOT = mybir.AluOpType

    def allreduce(cin, cout):
        if collectives:
            nc.gpsimd.collective_compute(
                "AllReduce", AOT.add, replica_groups=rg,
                ins=[cin[:]], outs=[cout[:]])
        else:
            nc.sync.dma_start(cout[:], cin[:])

    def bn_scale_shift(pool, pfx, d, gsums, gsumsq, n_all, g_in, beta_in):
        """-> (scale, shift) [d,1] tiles: scale = g*rsqrt(var+eps),
        shift = beta - mean*scale."""
        mean = pool.tile([d, 1], F32, tag=f"{pfx}mean")
        var = pool.tile([d, 1], F32, tag=f"{pfx}var")
        nc.vector.tensor_scalar_mul(mean[:], gsums[:], 1.0 / n_all)
        nc.vector.tensor_scalar_mul(var[:], gsumsq[:], 1.0 / n_all)
        m2 = pool.tile([d, 1], F32, tag=f"{pfx}m2")
        nc.vector.tensor_mul(m2[:], mean[:], mean[:])
        nc.vector.tensor_sub(var[:], var[:], m2[:])
        nc.vector.tensor_scalar_add(var[:], var[:], EPS)
        r0 = pool.tile([d, 1], F32, tag=f"{pfx}r0")
        nc.vector.reciprocal(r0[:], var[:])
        rst = pool.tile([d, 1], F32, tag=f"{pfx}rst")
        nc.scalar.activation(rst[:], r0[:], AF.Sqrt)
        t0 = pool.tile([d, 1], F32, tag=f"{pfx}t0")
        nc.vector.tensor_mul(t0[:], rst[:], rst[:])
        nc.vector.tensor_mul(t0[:], t0[:], var[:])
        nc.vector.tensor_scalar(t0[:], t0[:], -0.5, 1.5, op0=AOT.mult, op1=AOT.add)
        nc.vector.tensor_mul(rst[:], rst[:], t0[:])
        gsb = pool.tile([d, 1], F32, tag=f"{pfx}gsb")
        bsb = pool.tile([d, 1], F32, tag=f"{pfx}bsb")
        nc.sync.dma_start(gsb[:], g_in[:])
        nc.sync.dma_start(bsb[:], beta_in[:])
        scale = pool.tile([d, 1], F32, tag=f"{pfx}scale")
        nc.vector.tensor_mul(scale[:], rst[:], gsb[:])
        shift = pool.tile([d, 1], F32, tag=f"{pfx}shift")
        nc.vector.tensor_mul(shift[:], mean[:], scale[:])
        nc.vector.tensor_sub(shift[:], bsb[:], shift[:])
        return scale, shift

    with TileContext(nc) as tc:
        with tc.tile_pool(name="persist", bufs=1) as pp:
            idx16 = pp.tile([128, 3 * n_qt], I16, tag="idx16")

            # ================= phase A: stage features (1,2,3 then 0) ======
            def stage_feats(s, src, npts, n_all, w_in, g_in, beta_in, dst,
                            fpool, partial_cc=None):
                fdim = FDIMS[s]
                nk = max(1, fdim // 128)
                kp = fdim // nk
                nch = npts // CH
                with tc.tile_pool(name=f"stA{s}", bufs=3) as sp, \
                     tc.tile_pool(name=f"stApsum{s}", bufs=4, space="PSUM") as psp:
                    wt = []
                    for k in range(nk):
                        w = sp.tile([kp, DOUT], F32, tag="w")
                        nc.sync.dma_start(w[:], w_in[k * kp:(k + 1) * kp, :])
                        wt.append(w)
                    fsb = fpool.tile([DOUT, npts], F32, tag=f"fsb{s}")
                    scol = sp.tile([DOUT, nch], F32, tag="scol")
                    sqcol = sp.tile([DOUT, nch], F32, tag="sqcol")
                    for ci in range(nch):
                        ps = psp.tile([DOUT, CH], F32, tag="ps")
                        for k in range(nk):
                            xt = sp.tile([kp, CH], F32, tag="xt")
                            nc.sync.dma_start(
                                xt[:], src[k * kp:(k + 1) * kp,
                                           ci * CH:(ci + 1) * CH])
                            nc.tensor.matmul(ps[:], wt[k][:], xt[:],
                                             start=(k == 0), stop=(k == nk - 1))
                        nc.scalar.activation(
                            fsb[:, ci * CH:(ci + 1) * CH], ps[:], AF.Copy,
                            accum_out=scol[:, ci:ci + 1])
                        sq = sp.tile([DOUT, CH], F32, tag="sq")
                        nc.scalar.activation(
                            sq[:], ps[:], AF.Square,
                            accum_out=sqcol[:, ci:ci + 1])
                    sums = sp.tile([DOUT, 1], F32, tag="sums")
                    sumsq = sp.tile([DOUT, 1], F32, tag="sumsq")
                    nc.vector.reduce_sum(sums[:], scol[:], axis=ax)
                    nc.vector.reduce_sum(sumsq[:], sqcol[:], axis=ax)

                    def finish(gsums, gsumsq, fin_pool):
                        scale, shift = bn_scale_shift(
                            fin_pool, f"s{s}", DOUT, gsums, gsumsq, n_all,
                            g_in, beta_in)
                        nc.scalar.activation(fsb[:], fsb[:], AF.Relu,
                                             bias=shift[:], scale=scale[:])
                        nc.sync.dma_start(dst[:], fsb[:])

                    if partial_cc is None:
                        with tc.tile_pool(name=f"stAF{s}", bufs=1) as fp2:
                            finish(sums, sumsq, fp2)
                        return None
                    nc.sync.dma_start(partial_cc[0, 0:DOUT], sums[:])
                    nc.sync.dma_start(partial_cc[0, DOUT:2 * DOUT], sumsq[:])
                    return finish

            # stages 1..3 replicated, local stats; each scope frees its fsb
            for s in range(3):
                with tc.tile_pool(name=f"fsbp{s+1}", bufs=1) as fpool:
                    stage_feats(s + 1, xT[s], ms[s], ms[s], sw[s + 1],
                                sg[s + 1], sbeta[s + 1], fd[s], fpool)
            # stage 0 (sharded): partial stats + allreduce, then finish
            with tc.tile_pool(name="fsbp0", bufs=1) as fpool0:
                fin0 = stage_feats(0, x0T, qpc, n0_full, sw[0], sg[0],
                                   sbeta[0], f0d, fpool0, partial_cc=cc0in)
                allreduce(cc0in, cc0out)
                with tc.tile_pool(name="st0fin", bufs=1) as zp:
                    gs = zp.tile([DOUT, 1], F32, tag="gs")
                    gq = zp.tile([DOUT, 1], F32, tag="gq")
                    nc.sync.dma_start(
                        gs[:], bass.AP(cc0out.tensor, 0, [[1, DOUT], [1, 1]]))
                    nc.sync.dma_start(
                        gq[:], bass.AP(cc0out.tensor, DOUT, [[1, DOUT], [1, 1]]))
                    fin0(gs, gq, zp)

            # ================= phase B: kNN argmin ==========================
            m23 = ms[1] + ms[2]
            with tc.tile_pool(name="tbl", bufs=1) as tp:
                caug_sb = []
                for s in range(3):
                    t = tp.tile([rows(packs[s]), ms[s] // packs[s]], F32,
                                tag=f"caug{s}")
                    nc.sync.dma_start(t[:], caug_in[s][:])
                    caug_sb.append(t)
                # rows 32g+3..32g+5 arrive as copies of xyz; square via a
                # scratch tile (engine ops can't start at partition 32g+3)
                wmax = max(ms[s] // packs[s] for s in range(3))
                with tc.tile_pool(name="sqscr", bufs=1) as qsp:
                    scr = qsp.tile([128, wmax], F32, tag="scr")
                    for s in range(3):
                        w = ms[s] // packs[s]
                        for g in range(packs[s]):
                            nc.scalar.activation(
                                scr[32 * g:32 * g + 3, :w],
                                caug_sb[s][32 * g:32 * g + 3, :], AF.Square)
                            nc.sync.dma_start(
                                caug_sb[s][32 * g + 3:32 * g + 6, :],
                                scr[32 * g:32 * g + 3, :w])

                maxp = max(packs)
                with tc.tile_pool(name="knn", bufs=1) as kp_, \
                     tc.tile_pool(name="stg", bufs=2) as sgp, \
                     tc.tile_pool(name="knnpsum", bufs=2, space="PSUM") as kpp:
                    vals1 = kp_.tile([128, ms[0]], F32, tag="vals1")
                    vals23 = kp_.tile([128, m23], F32, tag="vals23")
                    idxbuf = kp_.tile([128, 3 * n_qt], F32, tag="idxbuf")
                    for qt in range(n_qt):
                        # stage the query tile at partition bases 0/32/64/96
                        stg = sgp.tile([32 * (maxp - 1) + 6, 128], F32, tag="stg")
                        for g in range(maxp):
                            nc.sync.dma_start(
                                stg[32 * g:32 * g + 6, :],
                                qaug_in[0:6, qt * 128:(qt + 1) * 128])
                        for s in range(3):
                            p = packs[s]
                            w = ms[s] // p
                            tpg = w // CH
                            base = 0 if s < 2 else ms[1]
                            dv = vals1 if s == 0 else vals23
                            if p > 1:
                                # p group-matmuls -> one psum tile -> one
                                # strided ACT copy into global-order columns
                                for t in range(tpg):
                                    ps = kpp.tile([128, p * CH], F32, tag="ps")
                                    for g in range(p):
                                        nc.tensor.matmul(
                                            ps[:, g * CH:(g + 1) * CH],
                                            stg[32 * g:32 * g + 6, :],
                                            caug_sb[s][32 * g:32 * g + 6,
                                                       t * CH:(t + 1) * CH],
                                            start=True, stop=True,
                                            tile_position=(32 * g, 0))
                                    dst = bass.AP(
                                        dv.tensor,
                                        dv.offset + base + t * CH,
                                        [dv.ap[0], [w, p], [1, CH]])
                                    nc.scalar.activation(dst, ps[:], AF.Copy)
                            else:
                                # batch consecutive tiles, contiguous copy
                                bt = 4
                                for t0 in range(0, tpg, bt):
                                    nb = min(bt, tpg - t0)
                                    ps = kpp.tile([128, nb * CH], F32, tag="ps")
                                    for j in range(nb):
                                        nc.tensor.matmul(
                                            ps[:, j * CH:(j + 1) * CH],
                                            stg[0:6, :],
                                            caug_sb[s][0:6,
                                                       (t0 + j) * CH:
                                                       (t0 + j + 1) * CH],
                                            start=True, stop=True,
                                            tile_position=(0, 0))
                                    co = base + t0 * CH
                                    nc.scalar.activation(
                                        dv[:, co:co + nb * CH], ps[:], AF.Copy)
                        views = [vals1[:, :],
                                 vals23[:, 0:ms[1]],
                                 vals23[:, ms[1]:m23]]
                        for s in range(3):
                            acc = kp_.tile([128, 1], F32, tag=f"acc{s}")
                            v = views[s]
                            nc.vector._custom_dve(argmax_op, out=v[:, ::-1],
                                                  in0=v[:, ::-1],
                                                  accum_out=acc[:])
                            nc.vector.tensor_scalar(
                                idxbuf[:, s * n_qt + qt:s * n_qt + qt + 1],
                                acc[:], -1.0, float(ms[s] - 1),
                                op0=AOT.mult, op1=AOT.add)
                    nc.vector.tensor_copy(idx16[:], idxbuf[:])

            # ================= phase C: gather + classifier =================
            for s in range(3):
                nc.sync.dma_start(
                    bass.AP(idxd[s].tensor, 0, [[1, 128], [128, n_qt]]),
                    idx16[:, s * n_qt:(s + 1) * n_qt])
            for s in range(3):
                with tc.tile_pool(name=f"gat{s}", bufs=1) as gp:
                    fsb = gp.tile([DOUT, ms[s]], F32, tag="gfsb")
                    nc.sync.dma_start(fsb[:], fd[s][:])
                    wr = gp.tile([DOUT, qpc // 16], I16, tag="wr")
                    for k in range(4):
                        nc.sync.dma_start(
                            wr[16 * k:16 * k + 16, :],
                            bass.AP(idxd[s].tensor, 0, [[1, 16], [16, qpc // 16]]))
                    gsb = gp.tile([DOUT, qpc], F32, tag="ggsb")
                    nc.gpsimd.ap_gather(gsb[:], fsb[:], wr[:],
                                        channels=DOUT, num_elems=ms[s],
                                        d=1, num_idxs=qpc)
                    nc.sync.dma_start(gd[s][:], gsb[:])

            nchq = qpc // CH
            with tc.tile_pool(name="cls", bufs=3) as cp, \
                 tc.tile_pool(name="clspsum", bufs=4, space="PSUM") as cpp:
                w1t = []
                for k in range(2):
                    w = cp.tile([128, DCLS], F32, tag="w1t")
                    nc.sync.dma_start(w[:], cw1[k * 128:(k + 1) * 128, :])
                    w1t.append(w)
                hsb = cp.tile([DCLS, qpc], F32, tag="hsb")
                scol = cp.tile([DCLS, nchq], F32, tag="cscol")
                sqcol = cp.tile([DCLS, nchq], F32, tag="csqcol")
                for ci in range(nchq):
                    ps = cpp.tile([DCLS, CH], F32, tag="cps")
                    for k in range(2):
                        xt = cp.tile([128, CH], F32, tag="cxt")
                        lo, hi = (f0d, gd[0]) if k == 0 else (gd[1], gd[2])
                        nc.sync.dma_start(xt[0:64, :],
                                          lo[:, ci * CH:(ci + 1) * CH])
                        nc.sync.dma_start(xt[64:128, :],
                                          hi[:, ci * CH:(ci + 1) * CH])
                        nc.tensor.matmul(ps[:], w1t[k][:], xt[:],
                                         start=(k == 0), stop=(k == 1))
                    nc.scalar.activation(hsb[:, ci * CH:(ci + 1) * CH], ps[:],
                                         AF.Copy, accum_out=scol[:, ci:ci + 1])
                    sq = cp.tile([DCLS, CH], F32, tag="csq")
                    nc.scalar.activation(sq[:], ps[:], AF.Square,
                                         accum_out=sqcol[:, ci:ci + 1])
                csums = cp.tile([DCLS, 1], F32, tag="csums")
                csumsq = cp.tile([DCLS, 1], F32, tag="csumsq")
                nc.vector.reduce_sum(csums[:], scol[:], axis=ax)
                nc.vector.reduce_sum(csumsq[:], sqcol[:], axis=ax)
                nc.sync.dma_start(cc1in[0, 0:DCLS], csums[:])
                nc.sync.dma_start(cc1in[0, DCLS:2 * DCLS], csumsq[:])
                allreduce(cc1in, cc1out)
                gs = cp.tile([DCLS, 1], F32, tag="cgs")
                gq = cp.tile([DCLS, 1], F32, tag="cgq")
                nc.sync.dma_start(gs[:], bass.AP(cc1out.tensor, 0,
                                                 [[1, DCLS], [1, 1]]))
                nc.sync.dma_start(gq[:], bass.AP(cc1out.tensor, DCLS,
                                                 [[1, DCLS], [1, 1]]))
                scale, shift = bn_scale_shift(cp, "c", DCLS, gs, gq, n0_full,
                                              cg1, cbeta1)
                nc.scalar.activation(hsb[:], hsb[:], AF.Relu,
                                     bias=shift[:], scale=scale[:])
                w2t = cp.tile([DCLS, KCLS], F32, tag="w2t")
                nc.sync.dma_start(w2t[:], cw2[:])
                b2t = cp.tile([128, KCLS], F32, tag="b2t")
                nc.sync.dma_start(b2t[:], b2rep[:])
                for qt in range(n_qt):
                    po = cpp.tile([128, KCLS], F32, tag="po")
                    nc.tensor.matmul(po[:], hsb[:, qt * 128:(qt + 1) * 128],
                                     w2t[:], start=True, stop=True)
                    ot = cp.tile([128, KCLS], F32, tag="ot")
                    nc.vector.tensor_add(ot[:], po[:], b2t[:])
                    nc.sync.dma_start(out_d[qt * 128:(qt + 1) * 128, :], ot[:])

    nc.compile()
    return nc


# ---------------------------------------------------------------- host side
def prep_inmaps(inputs, qpc=N0_FULL // N_CORES, ms=tuple(MS)):
    ms = list(ms)
    p0 = np.asarray(inputs["p0"], np.float32)
    x0 = np.asarray(inputs["x0"], np.float32)
    ps = [np.asarray(inputs[f"p{s+1}"], np.float32) for s in range(3)]
    xs = [np.asarray(inputs[f"x{s+1}"], np.float32) for s in range(3)]
    sw = [np.asarray(w, np.float32) for w in inputs["stage_w"]]
    sg = [np.asarray(w, np.float32).reshape(-1, 1) for w in inputs["stage_g"]]
    sbeta = [np.asarray(w, np.float32).reshape(-1, 1)
             for w in inputs["stage_beta"]]
    w1 = np.asarray(inputs["w1"], np.float32)
    g1 = np.asarray(inputs["g1"], np.float32).reshape(-1, 1)
    beta1 = np.asarray(inputs["beta1"], np.float32).reshape(-1, 1)
    w2 = np.asarray(inputs["w2"], np.float32)
    b2 = np.asarray(inputs["b2"], np.float32)

    packs = [_pack_factor(m) for m in ms]
    qpack = _pack_factor(qpc, unit=128)

    def pack6(a6, p):
        # [6, M] -> [32(p-1)+6, M/p]; group g's 6 rows at partition 32g
        m = a6.shape[1]
        w = m // p
        z = np.zeros((32 * (p - 1) + 6, w), np.float32)
        for g in range(p):
            z[32 * g:32 * g + 6, :] = a6[:, g * w:(g + 1) * w]
        return z

    shared = {}
    for s in range(3):
        a = np.concatenate([ps[s].T, ps[s].T], axis=0).astype(np.float32)
        shared[f"caug{s}"] = pack6(a, packs[s])
        shared[f"x{s+1}T"] = np.ascontiguousarray(xs[s].T)
    for i in range(4):
        shared[f"sw{i}"] = sw[i]
        shared[f"sg{i}"] = sg[i]
        shared[f"sbeta{i}"] = sbeta[i]
    shared["cw1"] = w1
    shared["cg1"] = g1
    shared["cbeta1"] = beta1
    shared["cw2"] = w2
    shared["b2rep"] = np.ascontiguousarray(
        np.broadcast_to(b2.reshape(1, -1), (128, b2.shape[0])))

    in_maps = []
    for c in range(N_CORES):
        sl = slice(c * qpc, (c + 1) * qpc)
        m = dict(shared)
        m["qaug"] = np.ascontiguousarray(np.concatenate(
            [p0[sl].T, np.full((3, qpc), -0.5, np.float32)], axis=0))
        m["x0T"] = np.ascontiguousarray(x0[sl].T)
        in_maps.append(m)
    return in_maps


_NC_CACHE = {}


def kernel(**inputs):
    key = "full"
    if key not in _NC_CACHE:
        _NC_CACHE[key] = build_nc()
    nc = _NC_CACHE[key]
    in_maps = prep_inmaps(inputs)
    res = run_bass_kernel_spmd(nc, in_maps, list(range(N_CORES)))
    out = np.concatenate([res.results[c]["out"] for c in range(N_CORES)], axis=0)
    return np.ascontiguousarray(out.astype(np.float32))


# revision 16
# speedup vs baseline: 1.6750x; 1.1472x over previous
"""Trainium2 Bass kernel for nn_MultiHead (retrieval_knn).

Full inputs in, full output out. Internally: shard the N0 query axis across
8 NeuronCores, replicate the coarse stages; BN batch stats all-reduced.
"""

import numpy as np

import concourse.bass as bass
import concourse.bacc as bacc
import concourse.mybir as mybir
from concourse.tile import TileContext
from concourse.bass_utils import run_bass_kernel_spmd
from concourse.dve_spec import Spec, Src0, select, eq, scan, Idx, AluOp, lower, MaxNeg
import concourse.dve_ops as dve_ops
from concourse.dve_ops import DveOp, OPS, CUSTOM_DVE_SPECS
from concourse.dve_uop import DveOpSpec

F32 = mybir.dt.float32
I16 = mybir.dt.int16
AF = mybir.ActivationFunctionType

N_CORES = 8
N0_FULL = 98304
MS = [24576, 6144, 1536]          # candidate counts, stages 1..3
FDIMS = [64, 128, 256, 512]       # encoder dims, stages 0..3
DOUT = 64
DCLS = 128
KCLS = 17
EPS = 1e-5
CH = 512                          # matmul moving chunk


# ---------------------------------------------------------------- custom DVE op
def _register_argmax_op():
    """Fused single-pass argmax: feed the stream REVERSED; accum_out is the
    reversed-stream position of the last max attainment == first forward
    occurrence of the max.  idx_fwd = (N-1) - accum."""
    for op in OPS:
        if op.name == "ARGMAX_REV_ANT":
            return op

    FLT_MAX = np.float32(3.4028235e38)

    def _ref(in0, in1, s0, s1, imm2):
        r = np.maximum.accumulate(in0, axis=-1)
        out = np.where(
            in0 == r, np.arange(in0.shape[-1], dtype=np.float32), -FLT_MAX
        )
        acc = out.max(axis=-1, keepdims=True)
        return out, acc

    spec = Spec(
        body=select(eq(scan(AluOp.MAX, Src0), Src0), Idx, MaxNeg),
        accum=AluOp.MAX,
        reference=_ref,
    )
    op = DveOp("ARGMAX_REV_ANT", spec, subdim=False, uops_sha={})
    for ver in ("v3", "v4"):
        s = DveOpSpec(name=op.name, opcode=0, uops=lower(spec, ver=ver), rd1_en=False)
        op.uops_sha[ver] = s.sha(ver)
    OPS.append(op)
    CUSTOM_DVE_SPECS[op.name] = spec
    dve_ops._SUB_OPCODE_FOR_NAME[op.name] = dve_ops._CUSTOM_DVE_ROW_BASE + len(OPS) - 1
    assert max(dve_ops._SUB_OPCODE_FOR_NAME.values()) < 0x20
    return op


def _pack_factor(m, unit=CH):
    # pack [6, M] -> groups of 6 rows at partition 32*g, so the SBUF
    # byte-range allocation stays small; engine ops must start at 0/32/64/96
    for p in (4, 2, 1):
        if m % p == 0 and (m // p) % unit == 0:
            return p
    return 1


# ---------------------------------------------------------------- builder
def build_nc(qpc=N0_FULL // N_CORES, ms=tuple(MS), n0_full=N0_FULL,
             collectives=True):
    """Build the SPMD Bass program for one core (same NEFF on all cores)."""
    argmax_op = _register_argmax_op()
    ms = list(ms)
    n_qt = qpc // 128
    assert qpc % CH == 0 and qpc % 128 == 0
    for m in ms:
        assert m % CH == 0

    packs = [_pack_factor(m) for m in ms]
    qpack = _pack_factor(qpc, unit=128)
    qw = qpc // qpack
    assert qw % 128 == 0

    def rows(p):
        return 32 * (p - 1) + 6

    nc = bacc.Bacc("TRN2", target_bir_lowering=False, debug=False,
                   num_devices=N_CORES)

    # ---- I/O -------------------------------------------------------------
    def din(name, shape):
        return nc.dram_tensor(name, shape, F32, kind="ExternalInput").ap()

    qaug_in = din("qaug", [6, qpc])
    caug_in = [din(f"caug{s}", [rows(packs[s]), ms[s] // packs[s]])
               for s in range(3)]
    x0T = din("x0T", [64, qpc])
    xT = [din(f"x{s+1}T", [FDIMS[s + 1], ms[s]]) for s in range(3)]
    sw = [din(f"sw{i}", [FDIMS[i], DOUT]) for i in range(4)]
    sg = [din(f"sg{i}", [DOUT, 1]) for i in range(4)]
    sbeta = [din(f"sbeta{i}", [DOUT, 1]) for i in range(4)]
    cw1 = din("cw1", [4 * DOUT, DCLS])
    cg1 = din("cg1", [DCLS, 1])
    cbeta1 = din("cbeta1", [DCLS, 1])
    cw2 = din("cw2", [DCLS, KCLS])
    b2rep = din("b2rep", [128, KCLS])

    out_d = nc.dram_tensor("out", [qpc, KCLS], F32, kind="ExternalOutput").ap()

    # ---- internal DRAM ---------------------------------------------------
    f0d = nc.dram_tensor("f0d", [DOUT, qpc], F32).ap()
    fd = [nc.dram_tensor(f"f{s+1}d", [DOUT, ms[s]], F32).ap() for s in range(3)]
    gd = [nc.dram_tensor(f"g{s+1}d", [DOUT, qpc], F32).ap() for s in range(3)]
    idxd = [nc.dram_tensor(f"idx{s+1}d", [qpc], I16).ap() for s in range(3)]
    cc0in = nc.dram_tensor("cc0in", [1, 2 * DOUT], F32).ap()
    cc0out = nc.dram_tensor("cc0out", [1, 2 * DOUT], F32, addr_space="Shared").ap()
    cc1in = nc.dram_tensor("cc1in", [1, 2 * DCLS], F32).ap()
    cc1out = nc.dram_tensor("cc1out", [1, 2 * DCLS], F32, addr_space="Shared").ap()

    rg = [list(range(N_CORES))]
    ax = mybir.AxisListType.X
    AOT = mybir.AluOpType

    def allreduce(cin, cout):
        if collectives:
            nc.gpsimd.collective_compute(
                "AllReduce", AOT.add, replica_groups=rg,
                ins=[cin[:]], outs=[cout[:]])
        else:
            nc.sync.dma_start(cout[:], cin[:])

    def bn_scale_shift(pool, pfx, d, gsums, gsumsq, n_all, g_in, beta_in):
        """-> (scale, shift) [d,1] tiles: scale = g*rsqrt(var+eps),
        shift = beta - mean*scale."""
        mean = pool.tile([d, 1], F32, tag=f"{pfx}mean")
        var = pool.tile([d, 1], F32, tag=f"{pfx}var")
        nc.vector.tensor_scalar_mul(mean[:], gsums[:], 1.0 / n_all)
        nc.vector.tensor_scalar_mul(var[:], gsumsq[:], 1.0 / n_all)
        m2 = pool.tile([d, 1], F32, tag=f"{pfx}m2")
        nc.vector.tensor_mul(m2[:], mean[:], mean[:])
        nc.vector.tensor_sub(var[:], var[:], m2[:])
        nc.vector.tensor_scalar_add(var[:], var[:], EPS)
        r0 = pool.tile([d, 1], F32, tag=f"{pfx}r0")
        nc.vector.reciprocal(r0[:], var[:])
        rst = pool.tile([d, 1], F32, tag=f"{pfx}rst")
        nc.scalar.activation(rst[:], r0[:], AF.Sqrt)
        t0 = pool.tile([d, 1], F32, tag=f"{pfx}t0")
        nc.vector.tensor_mul(t0[:], rst[:], rst[:])
        nc.vector.tensor_mul(t0[:], t0[:], var[:])
        nc.vector.tensor_scalar(t0[:], t0[:], -0.5, 1.5, op0=AOT.mult, op1=AOT.add)
        nc.vector.tensor_mul(rst[:], rst[:], t0[:])
        gsb = pool.tile([d, 1], F32, tag=f"{pfx}gsb")
        bsb = pool.tile([d, 1], F32, tag=f"{pfx}bsb")
        nc.sync.dma_start(gsb[:], g_in[:])
        nc.sync.dma_start(bsb[:], beta_in[:])
        scale = pool.tile([d, 1], F32, tag=f"{pfx}scale")
        nc.vector.tensor_mul(scale[:], rst[:], gsb[:])
        shift = pool.tile([d, 1], F32, tag=f"{pfx}shift")
        nc.vector.tensor_mul(shift[:], mean[:], scale[:])
        nc.vector.tensor_sub(shift[:], bsb[:], shift[:])
        return scale, shift

    with TileContext(nc) as tc:
        with tc.tile_pool(name="persist", bufs=1) as pp:
            idx16 = pp.tile([128, 3 * n_qt], I16, tag="idx16")

            # ================= phase A: stage features (1,2,3 then 0) ======
            def stage_feats(s, src, npts, n_all, w_in, g_in, beta_in, dst,
                            fpool, partial_cc=None):
                fdim = FDIMS[s]
                nk = max(1, fdim // 128)
                kp = fdim // nk
                nch = npts // CH
                with tc.tile_pool(name=f"stA{s}", bufs=3) as sp, \
                     tc.tile_pool(name=f"stApsum{s}", bufs=4, space="PSUM") as psp:
                    wt = []
                    for k in range(nk):
                        w = sp.tile([kp, DOUT], F32, tag="w")
                        nc.sync.dma_start(w[:], w_in[k * kp:(k + 1) * kp, :])
                        wt.append(w)
                    fsb = fpool.tile([DOUT, npts], F32, tag=f"fsb{s}")
                    scol = sp.tile([DOUT, nch], F32, tag="scol")
                    sqcol = sp.tile([DOUT, nch], F32, tag="sqcol")
                    for ci in range(nch):
                        ps = psp.tile([DOUT, CH], F32, tag="ps")
                        for k in range(nk):
                            xt = sp.tile([kp, CH], F32, tag="xt")
                            nc.sync.dma_start(
                                xt[:], src[k * kp:(k + 1) * kp,
                                           ci * CH:(ci + 1) * CH])
                            nc.tensor.matmul(ps[:], wt[k][:], xt[:],
                                             start=(k == 0), stop=(k == nk - 1))
                        nc.scalar.activation(
                            fsb[:, ci * CH:(ci + 1) * CH], ps[:], AF.Copy,
                            accum_out=scol[:, ci:ci + 1])
                        sq = sp.tile([DOUT, CH], F32, tag="sq")
                        nc.scalar.activation(
                            sq[:], ps[:], AF.Square,
                            accum_out=sqcol[:, ci:ci + 1])
                    sums = sp.tile([DOUT, 1], F32, tag="sums")
                    sumsq = sp.tile([DOUT, 1], F32, tag="sumsq")
                    nc.vector.reduce_sum(sums[:], scol[:], axis=ax)
                    nc.vector.reduce_sum(sumsq[:], sqcol[:], axis=ax)

                    def finish(gsums, gsumsq, fin_pool):
                        scale, shift = bn_scale_shift(
                            fin_pool, f"s{s}", DOUT, gsums, gsumsq, n_all,
                            g_in, beta_in)
                        nc.scalar.activation(fsb[:], fsb[:], AF.Relu,
                                             bias=shift[:], scale=scale[:])
                        nc.sync.dma_start(dst[:], fsb[:])

                    if partial_cc is None:
                        with tc.tile_pool(name=f"stAF{s}", bufs=1) as fp2:
                            finish(sums, sumsq, fp2)
                        return None
                    nc.sync.dma_start(partial_cc[0, 0:DOUT], sums[:])
                    nc.sync.dma_start(partial_cc[0, DOUT:2 * DOUT], sumsq[:])
                    return finish

            # stages 1..3 replicated, local stats; each scope frees its fsb
            for s in range(3):
                with tc.tile_pool(name=f"fsbp{s+1}", bufs=1) as fpool:
                    stage_feats(s + 1, xT[s], ms[s], ms[s], sw[s + 1],
                                sg[s + 1], sbeta[s + 1], fd[s], fpool)
            # stage 0 (sharded): partial stats + allreduce, then finish
            with tc.tile_pool(name="fsbp0", bufs=1) as fpool0:
                fin0 = stage_feats(0, x0T, qpc, n0_full, sw[0], sg[0],
                                   sbeta[0], f0d, fpool0, partial_cc=cc0in)
                allreduce(cc0in, cc0out)
                with tc.tile_pool(name="st0fin", bufs=1) as zp:
                    gs = zp.tile([DOUT, 1], F32, tag="gs")
                    gq = zp.tile([DOUT, 1], F32, tag="gq")
                    nc.sync.dma_start(
                        gs[:], bass.AP(cc0out.tensor, 0, [[1, DOUT], [1, 1]]))
                    nc.sync.dma_start(
                        gq[:], bass.AP(cc0out.tensor, DOUT, [[1, DOUT], [1, 1]]))
                    fin0(gs, gq, zp)

            # ================= phase B: kNN argmin ==========================
            m23 = ms[1] + ms[2]
            with tc.tile_pool(name="tbl", bufs=1) as tp:
                caug_sb = []
                for s in range(3):
                    t = tp.tile([rows(packs[s]), ms[s] // packs[s]], F32,
                                tag=f"caug{s}")
                    nc.sync.dma_start(t[:], caug_in[s][:])
                    caug_sb.append(t)
                # rows 32g+3..32g+5 arrive as copies of xyz; square via a
                # scratch tile (engine ops can't start at partition 32g+3)
                wmax = max(ms[s] // packs[s] for s in range(3))
                with tc.tile_pool(name="sqscr", bufs=1) as qsp:
                    scr = qsp.tile([128, wmax], F32, tag="scr")
                    for s in range(3):
                        w = ms[s] // packs[s]
                        for g in range(packs[s]):
                            nc.scalar.activation(
                                scr[32 * g:32 * g + 3, :w],
                                caug_sb[s][32 * g:32 * g + 3, :], AF.Square)
                            nc.sync.dma_start(
                                caug_sb[s][32 * g + 3:32 * g + 6, :],
                                scr[32 * g:32 * g + 3, :w])

                maxp = max(packs)
                assert ms[2] * 4 <= 8192, "stage3 must fit psum"
                with tc.tile_pool(name="knn", bufs=1) as kp_, \
                     tc.tile_pool(name="stg", bufs=3) as sgp, \
                     tc.tile_pool(name="knnpsum", bufs=2, space="PSUM") as kpp:
                    vals1 = kp_.tile([128, ms[0]], F32, tag="vals1")
                    vals2 = kp_.tile([128, ms[1]], F32, tag="vals2")
                    idxbuf = kp_.tile([128, 3 * n_qt], F32, tag="idxbuf")

                    def scan_idx(view, s, qt):
                        acc = kp_.tile([128, 1], F32, tag=f"acc{s}")
                        nc.vector._custom_dve(argmax_op, out=view[:, ::-1],
                                              in0=view[:, ::-1],
                                              accum_out=acc[:])
                        nc.vector.tensor_scalar(
                            idxbuf[:, s * n_qt + qt:s * n_qt + qt + 1],
                            acc[:], -1.0, float(ms[s] - 1),
                            op0=AOT.mult, op1=AOT.add)

                    for qt in range(n_qt):
                        # stage the query tile at partition bases 0/32/64/96
                        stg = sgp.tile([32 * (maxp - 1) + 6, 128], F32, tag="stg")
                        for g in range(maxp):
                            nc.sync.dma_start(
                                stg[32 * g:32 * g + 6, :],
                                qaug_in[0:6, qt * 128:(qt + 1) * 128])
                        # stage 3 first: matmuls into one psum tensor, scan
                        # directly from PSUM (no ACT copy, no SBUF buffer)
                        ps3 = kpp.tile([128, 2048], F32, tag="ps")
                        tp3 = ms[2] // CH
                        for j in range(tp3):
                            nc.tensor.matmul(
                                ps3[:, j * CH:(j + 1) * CH], stg[0:6, :],
                                caug_sb[2][0:6, j * CH:(j + 1) * CH],
                                start=True, stop=True, tile_position=(0, 0))
                        scan_idx(ps3[:, 0:ms[2]], 2, qt)
                        # stages 2 then 1: psum -> ACT copy -> SBUF -> scan
                        for s in (1, 0):
                            p = packs[s]
                            w = ms[s] // p
                            tpg = w // CH
                            dv = vals2 if s == 1 else vals1
                            if p > 1:
                                for t in range(tpg):
                                    ps = kpp.tile([128, p * CH], F32, tag="ps")
                                    for g in range(p):
                                        nc.tensor.matmul(
                                            ps[:, g * CH:(g + 1) * CH],
                                            stg[32 * g:32 * g + 6, :],
                                            caug_sb[s][32 * g:32 * g + 6,
                                                       t * CH:(t + 1) * CH],
                                            start=True, stop=True,
                                            tile_position=(32 * g, 0))
                                    dst = bass.AP(
                                        dv.tensor,
                                        dv.offset + t * CH,
                                        [dv.ap[0], [w, p], [1, CH]])
                                    nc.scalar.activation(dst, ps[:], AF.Copy)
                            else:
                                bt = 4
                                for t0 in range(0, tpg, bt):
                                    nb = min(bt, tpg - t0)
                                    ps = kpp.tile([128, nb * CH], F32, tag="ps")
                                    for j in range(nb):
                                        nc.tensor.matmul(
                                            ps[:, j * CH:(j + 1) * CH],
                                            stg[0:6, :],
                                            caug_sb[s][0:6,
                                                       (t0 + j) * CH:
                                                       (t0 + j + 1) * CH],
                                            start=True, stop=True,
                                            tile_position=(0, 0))
                                    nc.scalar.activation(
                                        dv[:, t0 * CH:t0 * CH + nb * CH],
                                        ps[:], AF.Copy)
                            scan_idx(dv[:, 0:ms[s]], s, qt)
                    nc.vector.tensor_copy(idx16[:], idxbuf[:])

            # ================= phase C: gather + classifier =================
            for s in range(3):
                nc.sync.dma_start(
                    bass.AP(idxd[s].tensor, 0, [[1, 128], [128, n_qt]]),
                    idx16[:, s * n_qt:(s + 1) * n_qt])
            for s in range(3):
                with tc.tile_pool(name=f"gat{s}", bufs=1) as gp:
                    fsb = gp.tile([DOUT, ms[s]], F32, tag="gfsb")
                    nc.sync.dma_start(fsb[:], fd[s][:])
                    wr = gp.tile([DOUT, qpc // 16], I16, tag="wr")
                    for k in range(4):
                        nc.sync.dma_start(
                            wr[16 * k:16 * k + 16, :],
                            bass.AP(idxd[s].tensor, 0, [[1, 16], [16, qpc // 16]]))
                    gsb = gp.tile([DOUT, qpc], F32, tag="ggsb")
                    nc.gpsimd.ap_gather(gsb[:], fsb[:], wr[:],
                                        channels=DOUT, num_elems=ms[s],
                                        d=1, num_idxs=qpc)
                    nc.sync.dma_start(gd[s][:], gsb[:])

            nchq = qpc // CH
            with tc.tile_pool(name="cls", bufs=3) as cp, \
                 tc.tile_pool(name="clspsum", bufs=4, space="PSUM") as cpp:
                w1t = []
                for k in range(2):
                    w = cp.tile([128, DCLS], F32, tag="w1t")
                    nc.sync.dma_start(w[:], cw1[k * 128:(k + 1) * 128, :])
                    w1t.append(w)
                hsb = cp.tile([DCLS, qpc], F32, tag="hsb")
                scol = cp.tile([DCLS, nchq], F32, tag="cscol")
                sqcol = cp.tile([DCLS, nchq], F32, tag="csqcol")
                for ci in range(nchq):
                    ps = cpp.tile([DCLS, CH], F32, tag="cps")
                    for k in range(2):
                        xt = cp.tile([128, CH], F32, tag="cxt")
                        lo, hi = (f0d, gd[0]) if k == 0 else (gd[1], gd[2])
                        nc.sync.dma_start(xt[0:64, :],
                                          lo[:, ci * CH:(ci + 1) * CH])
                        nc.sync.dma_start(xt[64:128, :],
                                          hi[:, ci * CH:(ci + 1) * CH])
                        nc.tensor.matmul(ps[:], w1t[k][:], xt[:],
                                         start=(k == 0), stop=(k == 1))
                    nc.scalar.activation(hsb[:, ci * CH:(ci + 1) * CH], ps[:],
                                         AF.Copy, accum_out=scol[:, ci:ci + 1])
                    sq = cp.tile([DCLS, CH], F32, tag="csq")
                    nc.scalar.activation(sq[:], ps[:], AF.Square,
                                         accum_out=sqcol[:, ci:ci + 1])
                csums = cp.tile([DCLS, 1], F32, tag="csums")
                csumsq = cp.tile([DCLS, 1], F32, tag="csumsq")
                nc.vector.reduce_sum(csums[:], scol[:], axis=ax)
                nc.vector.reduce_sum(csumsq[:], sqcol[:], axis=ax)
                nc.sync.dma_start(cc1in[0, 0:DCLS], csums[:])
                nc.sync.dma_start(cc1in[0, DCLS:2 * DCLS], csumsq[:])
                allreduce(cc1in, cc1out)
                gs = cp.tile([DCLS, 1], F32, tag="cgs")
                gq = cp.tile([DCLS, 1], F32, tag="cgq")
                nc.sync.dma_start(gs[:], bass.AP(cc1out.tensor, 0,
                                                 [[1, DCLS], [1, 1]]))
                nc.sync.dma_start(gq[:], bass.AP(cc1out.tensor, DCLS,
                                                 [[1, DCLS], [1, 1]]))
                scale, shift = bn_scale_shift(cp, "c", DCLS, gs, gq, n0_full,
                                              cg1, cbeta1)
                nc.scalar.activation(hsb[:], hsb[:], AF.Relu,
                                     bias=shift[:], scale=scale[:])
                w2t = cp.tile([DCLS, KCLS], F32, tag="w2t")
                nc.sync.dma_start(w2t[:], cw2[:])
                b2t = cp.tile([128, KCLS], F32, tag="b2t")
                nc.sync.dma_start(b2t[:], b2rep[:])
                for qt in range(n_qt):
                    po = cpp.tile([128, KCLS], F32, tag="po")
                    nc.tensor.matmul(po[:], hsb[:, qt * 128:(qt + 1) * 128],
                                     w2t[:], start=True, stop=True)
                    ot = cp.tile([128, KCLS], F32, tag="ot")
                    nc.vector.tensor_add(ot[:], po[:], b2t[:])
                    nc.sync.dma_start(out_d[qt * 128:(qt + 1) * 128, :], ot[:])

    nc.compile()
    return nc


# ---------------------------------------------------------------- host side
def prep_inmaps(inputs, qpc=N0_FULL // N_CORES, ms=tuple(MS)):
    ms = list(ms)
    p0 = np.asarray(inputs["p0"], np.float32)
    x0 = np.asarray(inputs["x0"], np.float32)
    ps = [np.asarray(inputs[f"p{s+1}"], np.float32) for s in range(3)]
    xs = [np.asarray(inputs[f"x{s+1}"], np.float32) for s in range(3)]
    sw = [np.asarray(w, np.float32) for w in inputs["stage_w"]]
    sg = [np.asarray(w, np.float32).reshape(-1, 1) for w in inputs["stage_g"]]
    sbeta = [np.asarray(w, np.float32).reshape(-1, 1)
             for w in inputs["stage_beta"]]
    w1 = np.asarray(inputs["w1"], np.float32)
    g1 = np.asarray(inputs["g1"], np.float32).reshape(-1, 1)
    beta1 = np.asarray(inputs["beta1"], np.float32).reshape(-1, 1)
    w2 = np.asarray(inputs["w2"], np.float32)
    b2 = np.asarray(inputs["b2"], np.float32)

    packs = [_pack_factor(m) for m in ms]
    qpack = _pack_factor(qpc, unit=128)

    def pack6(a6, p):
        # [6, M] -> [32(p-1)+6, M/p]; group g's 6 rows at partition 32g
        m = a6.shape[1]
        w = m // p
        z = np.zeros((32 * (p - 1) + 6, w), np.float32)
        for g in range(p):
            z[32 * g:32 * g + 6, :] = a6[:, g * w:(g + 1) * w]
        return z

    shared = {}
    for s in range(3):
        a = np.concatenate([ps[s].T, ps[s].T], axis=0).astype(np.float32)
        shared[f"caug{s}"] = pack6(a, packs[s])
        shared[f"x{s+1}T"] = np.ascontiguousarray(xs[s].T)
    for i in range(4):
        shared[f"sw{i}"] = sw[i]
        shared[f"sg{i}"] = sg[i]
        shared[f"sbeta{i}"] = sbeta[i]
    shared["cw1"] = w1
    shared["cg1"] = g1
    shared["cbeta1"] = beta1
    shared["cw2"] = w2
    shared["b2rep"] = np.ascontiguousarray(
        np.broadcast_to(b2.reshape(1, -1), (128, b2.shape[0])))

    in_maps = []
    for c in range(N_CORES):
        sl = slice(c * qpc, (c + 1) * qpc)
        m = dict(shared)
        m["qaug"] = np.ascontiguousarray(np.concatenate(
            [p0[sl].T, np.full((3, qpc), -0.5, np.float32)], axis=0))
        m["x0T"] = np.ascontiguousarray(x0[sl].T)
        in_maps.append(m)
    return in_maps


_NC_CACHE = {}


def kernel(**inputs):
    key = "full"
    if key not in _NC_CACHE:
        _NC_CACHE[key] = build_nc()
    nc = _NC_CACHE[key]
    in_maps = prep_inmaps(inputs)
    res = run_bass_kernel_spmd(nc, in_maps, list(range(N_CORES)))
    out = np.concatenate([res.results[c]["out"] for c in range(N_CORES)], axis=0)
    return np.ascontiguousarray(out.astype(np.float32))
